# revision 1
# baseline (speedup 1.0000x reference)
"""Trainium2 Bass kernel for nn_DiffeqSolver (RK4 ODE integration of a 2-layer
tanh MLP vector field), data-parallel over 8 NeuronCores.

Problem (hardcoded):
  S, B, D, H, T = 4, 512, 256, 1024, 64
  f(y) = tanh(y @ W1^T + b1) @ W2^T + b2
  RK4 scan over dts = diff(time_steps_to_predict), output [S, B, T, D].

Strategy:
  - Shard the S*B = 2048 trajectories into 8 blocks of R = 256 rows (one per
    core); MLP params replicated; no cross-core communication.
  - On-chip, everything lives transposed: state y^T [D, R] with D on
    partitions, so both matmuls chain with no transposes:
      mm1: h^T[H,R] = (W1^T)^T-as-lhsT @ u^T   (contraction K = D)
      tanh on ScalarE (bias b1 fused), output a^T [H, R]
      mm2: f^T[D,R] = (W2^T)-as-lhsT @ a^T     (contraction K = H)
  - RK4 state updates are fused DVE scalar_tensor_tensor ops reading PSUM
    directly:  u_next = (k_psum * c) + y.  The RK4 accumulator is folded
    into k4's PSUM group via an identity matmul so each step ends in a
    single DVE op before the PE restarts.
  - Matmul operands in float32r (TF32-like; 1 PE cycle/row at N=256, ~16x
    more accurate than bf16); fp32 PSUM accumulation; state fp32.
  - Per step, y^T is DMA'd to DRAM out [T-1, D, R]; host reassembles.

Measured on 8 axon-tunneled trn2 cores: HW exec ~886 us (repeat-delta
calibration; PE streaming floor for this mapping is ~876 us), L2 rel err
6.7e-5 vs the fp32 reference.
"""

import os
import numpy as np
import ml_dtypes

import concourse.bass as bass
import concourse.mybir as mybir
import concourse.tile as tile
from concourse import bacc, bass_utils

S, B, D, H, T = 4, 512, 256, 1024, 64
N_CORES = 8
P = 128
RT = S * B            # 2048 total trajectories
R = RT // N_CORES     # 256 per core
DO = D // P           # 2 partition-chunks of D
HO = H // P           # 8 partition-chunks of H

F32 = mybir.dt.float32
ALU = mybir.AluOpType
ACTF = mybir.ActivationFunctionType

# matmul operand mode: "bf16" | "f32" | "f32r"
# f32r (TF32-like rounded fp32) measures both faster (885us vs 990us) and
# ~16x more accurate (6.7e-5 vs 1.1e-3 L2 rel err) than bf16 on HW.
MM_MODE = os.environ.get("BASS_MM_MODE", "f32r")


def _mm_np_dtype(mode):
    return ml_dtypes.bfloat16 if mode == "bf16" else np.float32


def _mm_bir_dtype(mode):
    if mode == "bf16":
        return mybir.dt.bfloat16
    if mode == "f32r":
        # TF32-like rounded fp32: PE streams it at 1 cycle/row (N>=256);
        # every producer (DMA/DVE/ACT) must declare float32r output so the
        # BIR verifier sees rounded operands.
        return mybir.dt.float32r
    return mybir.dt.float32


def _mm_ap(ap, mode):
    return ap


def build_nc(dts, mode=MM_MODE, b1_nonzero=True, b2_nonzero=False,
             n_out_steps=None, ks_major=False, upool_bufs=3, apool_bufs=2,
             ps1_bufs=3, ps2_bufs=5, repeat=1, out_last_only=False,
             tanh_split=False):
    """Build the Bass module. `dts` are the per-step fp32 dt values (baked as
    immediates). Output tensor is [len(dts), D, R] (state after each step)."""
    n_steps = len(dts)
    if n_out_steps is None:
        n_out_steps = n_steps
    mm_dt = _mm_bir_dtype(mode)
    # u/a/state-for-matmul tiles in the matmul dtype, separate fp32 state
    cast_inputs = mode in ("bf16", "f32r")

    # Bacc (not raw Bass): its finalize() legalizes multi-wait instructions
    # into event semaphores, which TRN2 walrus codegen requires.
    nc = bacc.Bacc()
    y0T_d = nc.dram_tensor("y0T", [D, R], F32, kind="ExternalInput")
    w1T_d = nc.dram_tensor("w1T", [D, H], mm_dt, kind="ExternalInput")
    w2T_d = nc.dram_tensor("w2T", [H, D], mm_dt, kind="ExternalInput")
    b1_d = nc.dram_tensor("b1", [H], F32, kind="ExternalInput")
    b2_d = nc.dram_tensor("b2", [D], mm_dt, kind="ExternalInput")
    ident_d = nc.dram_tensor("ident", [P, P], mm_dt, kind="ExternalInput")
    out_steps = 1 if out_last_only else n_steps
    out_d = nc.dram_tensor("outT", [out_steps, D, R], F32,
                           kind="ExternalOutput")

    with tile.TileContext(nc) as tc:
        with (
            tc.tile_pool(name="consts", bufs=1) as consts,
            tc.tile_pool(name="state", bufs=1) as state,
            tc.tile_pool(name="upool", bufs=upool_bufs) as upool,
            tc.tile_pool(name="apool", bufs=apool_bufs) as apool,
            tc.tile_pool(name="accpool", bufs=2) as accpool,
            tc.tile_pool(name="ps1", bufs=ps1_bufs, space="PSUM") as ps1,
            tc.tile_pool(name="ps2", bufs=ps2_bufs, space="PSUM") as ps2,
        ):
            # ---- persistent constants ----
            w1T = consts.tile([P, DO, H], mm_dt, name="w1T_sb")
            nc.sync.dma_start(
                w1T[:], w1T_d.ap().rearrange("(do dp) h -> dp do h", dp=P)
            )
            w2T = consts.tile([P, HO, D], mm_dt, name="w2T_sb")
            nc.sync.dma_start(
                w2T[:], w2T_d.ap().rearrange("(ho hp) d -> hp ho d", hp=P)
            )
            b1sb = consts.tile([P, HO], F32, name="b1_sb")
            nc.sync.dma_start(
                b1sb[:], b1_d.ap().rearrange("(ho hp) -> hp ho", hp=P)
            )
            if b2_nonzero:
                b2sb = consts.tile([1, D], mm_dt, name="b2_sb")
                nc.sync.dma_start(b2sb[:], b2_d.ap()[None, :])
                ones = consts.tile([1, R], mm_dt, name="ones_sb")
                nc.vector.memset(ones[:], 1.0)
            ident = consts.tile([P, P], mm_dt, name="ident_sb")
            nc.sync.dma_start(ident[:], ident_d.ap())

            # ---- state ----
            yT = state.tile([P, DO, R], F32, name="yT_sb")
            nc.sync.dma_start(
                yT[:], y0T_d.ap().rearrange("(do dp) r -> dp do r", dp=P)
            )
            if cast_inputs:
                ybf = state.tile([P, DO, R], mm_dt, name="ybf_sb")
                for dc in range(DO):
                    nc.vector.tensor_copy(ybf[:, dc, :], yT[:, dc, :])
            else:
                ybf = yT

            def f_eval(u_sb, extra_rhs=None):
                """One MLP eval. u_sb: [P, DO, R] (mm dtype). Returns the two
                PSUM tiles holding f^T's d-chunks (fp32, bias b2 included).
                extra_rhs: optional [P, DO, R] tensor added into the result
                via an identity matmul (used to fold the RK4 accumulator into
                k4's PSUM so the step ends in one DVE op)."""
                aT = apool.tile([P, HO, R], mm_dt, tag="aT", name="aT_sb")
                # ks-major: all K-subtile-0 matmuls first. They only need
                # u chunk 0, giving the DVE ~850ns to produce chunk 1 while
                # the PE streams — hides the eval-transition latency.
                pshs = [ps1.tile([P, 2, R], F32, tag="psh", name="psh")
                        for _ in range(HO // 2)]
                if ks_major == "pair01":
                    # pairs 0-1 emit their ks0 matmuls first (428ns of PE
                    # work gated only on u chunk 0), covering the DVE latency
                    # of u chunk 1; pairs 2-3 stay pair-major.
                    order = [(0, 0, 0), (0, 0, 1), (0, 1, 0), (0, 1, 1),
                             (1, 0, 0), (1, 0, 1), (1, 1, 0), (1, 1, 1)]
                    order += [(ks, p, h) for p in (2, 3)
                              for h in range(2) for ks in range(DO)]
                elif ks_major == "hybrid":
                    # front-load 6 ks0 matmuls (~640ns of PE work needing only
                    # u chunk 0) to cover the DVE latency of u chunk 1, then
                    # finish groups in pair order so tanh fires early.
                    head = [(0, p, h) for p in range(3) for h in range(2)]
                    tail = []
                    for p in range(HO // 2):
                        for h in range(2):
                            if p >= 3:
                                tail.append((0, p, h))
                            tail.append((1, p, h))
                    order = head + tail
                elif ks_major:
                    order = [(ks, pair, half) for ks in range(DO)
                             for pair in range(HO // 2) for half in range(2)]
                else:
                    order = [(ks, pair, half) for pair in range(HO // 2)
                             for half in range(2) for ks in range(DO)]
                for ks, pair, half in order:
                    hc = pair * 2 + half
                    nc.tensor.matmul(
                        pshs[pair][:, half, :],
                        _mm_ap(w1T[:, ks, hc * P:(hc + 1) * P], mode),
                        _mm_ap(u_sb[:, ks, :], mode),
                        start=(ks == 0),
                        stop=(ks == DO - 1),
                    )
                for pair in range(HO // 2):
                    psh = pshs[pair]
                    if b1_nonzero:
                        # per-partition bias differs per h-chunk: one ACT op
                        # per chunk
                        for half in range(2):
                            hc = pair * 2 + half
                            nc.scalar.activation(
                                aT[:, hc, :],
                                psh[:, half, :],
                                ACTF.Tanh,
                                bias=b1sb[:, hc:hc + 1],
                            )
                    elif tanh_split:
                        # two [128, 256] ops: shorter latency on the chain
                        # gating mm2's last matmuls (each half fires as soon
                        # as its accumulation group completes)
                        for half in range(2):
                            nc.scalar.activation(
                                aT[:, 2 * pair + half, :],
                                psh[:, half, :],
                                ACTF.Tanh,
                            )
                    else:
                        # fused [128, 512] tanh over the whole PSUM bank
                        nc.scalar.activation(
                            aT[:, 2 * pair:2 * pair + 2, :],
                            psh[:],
                            ACTF.Tanh,
                        )
                ktiles = [ps2.tile([P, R], F32, tag="psf", name="psf")
                          for _ in range(DO)]
                # Interleave: chunk0 hs0..6, chunk1 hs0, chunk0 hs7 (tanh3-
                # gated MM lands ~960ns after mm1, past the last tanh), then
                # chunk1 hs1..7. Keeps chunk0's group-stop early so the DVE
                # state update overlaps chunk1's matmuls.
                mm2_order = [(0, hs) for hs in range(HO - 1)]
                mm2_order += [(1, 0), (0, HO - 1), (0, "extras")]
                mm2_order += [(1, hs) for hs in range(1, HO)]
                mm2_order += [(1, "extras")]
                n_extra = int(b2_nonzero) + int(extra_rhs is not None)
                remaining = {dc: HO + n_extra for dc in range(DO)}
                for dc, hs in mm2_order:
                    psf = ktiles[dc]
                    if hs == "extras":
                        if b2_nonzero:
                            remaining[dc] -= 1
                            nc.tensor.matmul(
                                psf[:],
                                _mm_ap(b2sb[:, dc * P:(dc + 1) * P], mode),
                                _mm_ap(ones[:], mode),
                                start=False,
                                stop=(remaining[dc] == 0),
                            )
                        if extra_rhs is not None:
                            remaining[dc] -= 1
                            nc.tensor.matmul(
                                psf[:],
                                _mm_ap(ident[:], mode),
                                _mm_ap(extra_rhs[:, dc, :], mode),
                                start=False,
                                stop=(remaining[dc] == 0),
                            )
                        continue
                    remaining[dc] -= 1
                    nc.tensor.matmul(
                        psf[:],
                        _mm_ap(w2T[:, hs, dc * P:(dc + 1) * P], mode),
                        _mm_ap(aT[:, hs, :], mode),
                        start=(hs == 0),
                        stop=(remaining[dc] == 0),
                    )
                return ktiles

            stt = nc.vector.scalar_tensor_tensor
            for t_rep in range(n_steps * repeat):
                t = t_rep // repeat
                is_out = (t_rep % repeat) == repeat - 1
                dt = float(dts[t])
                acc = accpool.tile([P, DO, R], F32, tag="acc", name="acc_sb")

                # DVE ordering rule: emit the u-updates (next eval's matmul
                # input — the PE critical path) before the acc bookkeeping,
                # so the PE restarts after one DVE op instead of two.
                k1 = f_eval(ybf)
                u2 = upool.tile([P, DO, R], mm_dt, tag="u", name="u2_sb")
                with tc.high_priority():
                    for dc in range(DO):
                        stt(u2[:, dc, :], k1[dc][:], dt / 2, yT[:, dc, :],
                            ALU.mult, ALU.add)
                for dc in range(DO):
                    nc.vector.tensor_copy(acc[:, dc, :], k1[dc][:])

                k2 = f_eval(u2)
                u3 = upool.tile([P, DO, R], mm_dt, tag="u", name="u3_sb")
                with tc.high_priority():
                    for dc in range(DO):
                        stt(u3[:, dc, :], k2[dc][:], dt / 2, yT[:, dc, :],
                            ALU.mult, ALU.add)
                for dc in range(DO):
                    stt(acc[:, dc, :], k2[dc][:], 2.0, acc[:, dc, :],
                        ALU.mult, ALU.add)

                k3 = f_eval(u3)
                u4 = upool.tile([P, DO, R], mm_dt, tag="u", name="u4_sb")
                with tc.high_priority():
                    for dc in range(DO):
                        stt(u4[:, dc, :], k3[dc][:], dt, yT[:, dc, :],
                            ALU.mult, ALU.add)
                for dc in range(DO):
                    stt(acc[:, dc, :], k3[dc][:], 2.0, acc[:, dc, :],
                        ALU.mult, ALU.add)
                if cast_inputs:
                    # k4's PSUM group absorbs acc via an identity matmul, so
                    # the PE needs acc in matmul dtype (off critical path).
                    acc_mm = upool.tile([P, DO, R], mm_dt, tag="accbf",
                                        name="accbf_sb")
                    for dc in range(DO):
                        nc.vector.tensor_copy(acc_mm[:, dc, :], acc[:, dc, :])
                else:
                    acc_mm = acc

                # k4 PSUM = f(u4) + acc  →  y' = y + dt/6 * PSUM: the step
                # ends in a single DVE op before the PE can restart.
                k4 = f_eval(u4, extra_rhs=acc_mm)
                if cast_inputs:
                    with tc.high_priority():
                        for dc in range(DO):
                            stt(ybf[:, dc, :], k4[dc][:], dt / 6, yT[:, dc, :],
                                ALU.mult, ALU.add)
                for dc in range(DO):
                    stt(yT[:, dc, :], k4[dc][:], dt / 6, yT[:, dc, :],
                        ALU.mult, ALU.add)
                if out_last_only:
                    is_out = t_rep == n_steps * repeat - 1
                if is_out:
                    t_out = 0 if out_last_only else t
                    nc.sync.dma_start(
                        out_d.ap()[t_out].rearrange(
                            "(do dp) r -> dp do r", dp=P),
                        yT[:],
                    )
    # run_bass_via_pjrt serializes nc without finalizing; Bacc needs its
    # compile() legalization passes (reg alloc, event-sem wait splitting).
    nc.finalize()
    return nc


_CACHE = {}


def _get_nc(dts_key, mode, b1_nonzero, b2_nonzero, n_steps):
    key = (dts_key, mode, b1_nonzero, b2_nonzero, n_steps)
    if key not in _CACHE:
        _CACHE[key] = build_nc(
            np.asarray(dts_key, dtype=np.float32), mode=mode,
            b1_nonzero=b1_nonzero, b2_nonzero=b2_nonzero,
        )
    return _CACHE[key]


def kernel(first_point, time_steps_to_predict, W1, b1, W2, b2,
           trace=False, mode=None):
    if mode is None:
        mode = MM_MODE
    first_point = np.asarray(first_point, dtype=np.float32)
    tsp = np.asarray(time_steps_to_predict, dtype=np.float32)
    W1 = np.asarray(W1, dtype=np.float32)
    b1 = np.asarray(b1, dtype=np.float32)
    W2 = np.asarray(W2, dtype=np.float32)
    b2 = np.asarray(b2, dtype=np.float32)

    dts = np.diff(tsp)
    n_steps = len(dts)
    b1_nonzero = bool(np.any(b1))
    b2_nonzero = bool(np.any(b2))
    nc = _get_nc(tuple(dts.tolist()), mode, b1_nonzero, b2_nonzero, n_steps)

    np_mm = _mm_np_dtype(mode)
    w1T = np.ascontiguousarray(W1.T).astype(np_mm)    # [D, H]
    w2T = np.ascontiguousarray(W2.T).astype(np_mm)    # [H, D]
    b2c = b2.astype(np_mm)

    ident_np = np.eye(P, dtype=np.float32).astype(np_mm)
    rows = first_point.reshape(RT, D)
    in_maps = []
    for c in range(N_CORES):
        y0T = np.ascontiguousarray(rows[c * R:(c + 1) * R].T)  # [D, R]
        in_maps.append({
            "y0T": y0T, "w1T": w1T, "w2T": w2T, "b1": b1, "b2": b2c,
            "ident": ident_np,
        })

    res = bass_utils.run_bass_kernel_spmd(
        nc, in_maps, list(range(N_CORES)), trace=trace,
    )

    t_pts = n_steps + 1
    out = np.empty((RT, t_pts, D), dtype=np.float32)
    out[:, 0, :] = rows
    for c in range(N_CORES):
        o = res.results[c]["outT"]                     # [n_steps, D, R]
        out[c * R:(c + 1) * R, 1:, :] = o.transpose(2, 0, 1)
    full = out.reshape(S, B, t_pts, D)

    if trace:
        kernel.last_results = res
    return full

